# revision 45
# baseline (speedup 1.0000x reference)
"""DHASPI level-loss kernel for 8 Trainium2 NeuronCores.

Data-parallel over the fused B*C row axis: each core gets 64 rows of x_env
(SBUF partitions 0-63) and 64 rows of y_env (partitions 64-127). The device
computes per-row energies of the 200 non-overlapping 960-sample blocks
(gcd(9600, 2880) — every overlapping loudness frame is a sum of 10 of
them); everything downstream of the block sums — frame energies, the
absolute/relative gating, lufs, the relu-diff loss — is a tiny [1024, 200]
float64 numpy epilogue on the host.

The 200 blocks per row are processed as variable-size work units (1-5
blocks) spread across all four instruction-issuing engines so DMA and
compute overlap fully (raw Bass, manual semaphores):

  route   DMA                    square                block reduction
  C (73b) SP HWDGE f32           ACT Square+accum_out (fused, per block)
  B (18b) Pool SWDGE cast->fp8   ACT Square+accum_out (fused, per block)
  E (89b) Pool SWDGE cast->bf16  DVE mult (bf16, 2x)   Pool accum-DMA fold
                                                       960->480, then one
                                                       DVE reduce_sum
  N (13b) Pool SWDGE cast->bf16  DVE mult (bf16, 2x)   one DVE reduce_sum

Design notes:
- The split sizes the four engine loads to come out nearly equal (~108 us
  each): SP moves f32, Pool halves/quarters its transfer cost by casting to
  bf16/fp8 during the DMA, ACT squares with the fused per-block accumulator,
  and DVE squares at the 2x bf16 rate then reduces with one reduce_sum,
  folded 960->480 beforehand on Pool via an accumulate-DMA for E units.
- fp8(e4m3) input is only used on ACT-fed B units (ACT squares in f32
  regardless of input dtype); measured end-to-end loss error ~1e-3 vs the
  2e-2 tolerance.
- Chained in-place DVE add-tree levels were measured to return stale data
  on the device stochastically (same-engine RAW through the DVE write
  pipe; CoreSim does not model it), so all block reductions are single
  reduce_sum instructions — a sequential reduce never outruns its
  producer — with Pool's accumulate-DMA fold halving the reduce width for
  most units.
- Engine instruction orders are static: the DVE stream weaves N units with
  the E square/finish pipeline, and the Pool stream is deadline-sorted
  against a no-stall relaxation so casts and folds arrive just ahead of
  their consumers.
"""

import numpy as np

import concourse.bass as bass
from concourse import mybir
from concourse.bass_utils import run_bass_kernel_spmd

# Problem constants (hardcoded; kernel.py must be self-contained)
B, C, T = 16, 32, 192000
N_CORES = 8
ROWS = B * C  # 512
RPC = ROWS // N_CORES  # 64 rows per core per tensor

FRAME = 9600
SHIFT = 2880
BLK = 960  # gcd(FRAME, SHIFT)
NBLK = T // BLK  # 200 block sums per row
NFRM = (T - FRAME) // SHIFT + 1  # 64 frames per row

UB = 5  # blocks per work unit
USAMP = UB * BLK  # 4800 samples
NU = NBLK // UB  # 40 units

EPS = 1e-8
ALPHA = 1e-4
GAMMA_A = -70.0

F32 = mybir.dt.float32
BF16 = mybir.dt.bfloat16
FP8 = mybir.dt.float8e4

# Per-route unit sizes in 960-blocks. Small first units cut pipeline fill;
# small last units shorten each engine's tail behind the final transfers.
_FMARGIN = 2500.0  # fold deadline margin ns
_FINGAP = 5300.0  # min ns between sqE_k end and finE_k
C_SIZES = [1, 2, 3] + [4] * 17 + [2, 2]  # 78 blocks, 22 units (SP f32 -> ACT fused)
B_SIZES = [2, 3, 5, 5, 5]  # 20 blocks, 5 units (Pool fp8 -> ACT fused)
# Block reduction is reduce_sum only (see docstring): E units fold
# 960->480 on Pool first, N units reduce the full 960 directly.
E_SIZES = [2, 3] + [5] * 16 + [2, 2]  # 89 blocks (Pool -> DVE sq -> fold -> red480)
N_SIZES = [5, 5, 3]  # 13 blocks (Pool -> DVE sq -> red960)

# Contiguous global block ranges per route: C | B | E | N covers 0..199.
def _spans(sizes, start):
    out = []
    for s in sizes:
        out.append((start, s))
        start += s
    return out, start

C_SPANS, _o = _spans(C_SIZES, 0)
B_SPANS, _o = _spans(B_SIZES, _o)
E_SPANS, _o = _spans(E_SIZES, _o)
N_SPANS, _o = _spans(N_SIZES, _o)
assert _o == NBLK

# SP stream: all C units in order; ACT consumes them in the same order.
SP_STREAM = [("C", i) for i in range(len(C_SIZES))]  # 20 units
# ACT stream: C units as they land, B units filling ACT's spare rate.
ACT_STREAM = (
    [("SP", 0), ("SP", 1), ("B", 0), ("SP", 2), ("SP", 3), ("SP", 4)]
    + [("B", 1)]
    + [("SP", 5), ("SP", 6), ("SP", 7)]
    + [("B", 2)]
    + [("SP", 8), ("SP", 9), ("SP", 10)]
    + [("B", 3)]
    + [("SP", 11), ("SP", 12), ("SP", 13), ("SP", 14)]
    + [("B", 4)]
    + [("SP", 15), ("SP", 16), ("SP", 17), ("SP", 18), ("SP", 19), ("SP", 20)]
    + [("SP", 21)]
)
# DVE stream: N units and the E pipeline woven together from the start —
# sqE early so Pool's folds can fire early; finE only once the fold
# round-trip (fold DMA + completion latency) has plausibly finished.
def _cost_dve(kind, k):
    if kind == "N":
        return 1540.0 * N_SIZES[k] + 200.0
    if kind == "sqE":
        return 524.0 * E_SIZES[k] + 100.0
    return 500.0 * E_SIZES[k] + 100.0


def _build_dve_stream():
    out = []
    ne, nn = len(E_SIZES), len(N_SIZES)
    t = 0.0
    sq_end = {}

    def emit(kind, k):
        nonlocal t
        out.append((kind, k))
        t += _cost_dve(kind, k)
        if kind == "sqE":
            sq_end[k] = t

    emit("sqE", 0)
    emit("sqE", 1)
    ei_sq, ei_fin, ni = 2, 0, 0
    while ei_sq < ne or ei_fin < ne or ni < nn:
        if ni < nn:
            emit("N", ni); ni += 1
        if ei_sq < ne:
            # esq ring: finE_k must precede sqE_{k+NESQ}
            if ei_sq - ei_fin >= 4:
                emit("finE", ei_fin); ei_fin += 1
            emit("sqE", ei_sq); ei_sq += 1
        # fold_k issues after sqE_k lands; needs ~1850 + 2800 to come back
        if ei_fin < ne and (
            (ei_sq >= ne and ni >= nn)
            or (
                ei_sq - ei_fin >= 3
                and ei_fin < ei_sq
                and t >= sq_end[ei_fin] + _FINGAP
            )
        ):
            emit("finE", ei_fin); ei_fin += 1
    return out

DVE_STREAM = _build_dve_stream()

# Pool stream: generated by deadline-sorting casts and folds against a
# no-stall relaxation of the fixed DVE/ACT streams.
def _build_pool_stream():
    LAT = 2800.0  # DMA completion latency (init + sem prop)
    # DVE op begin times under no stalls
    t = 3500.0
    dve_begin = {}
    for kind, k in DVE_STREAM:
        dve_begin[(kind, k)] = t
        t += _cost_dve(kind, k)
    # ACT unit begin times under no stalls
    t = 4100.0
    act_begin = {}
    for kind, k in ACT_STREAM:
        act_begin[(kind, k)] = t
        nb = C_SIZES[k] if kind == "SP" else B_SIZES[k]
        t += 1182.0 * nb + 30.0
    ops = []
    for k in range(len(N_SIZES)):
        ops.append((dve_begin[("N", k)] - LAT, ("cN", k)))
    for k in range(len(E_SIZES)):
        ops.append((dve_begin[("sqE", k)] - LAT, ("cE", k)))
        # fold released only after sqE_k; deadline is finE_k
        rel = dve_begin[("sqE", k)] + 524.0 * E_SIZES[k] + 400.0
        ops.append((max(dve_begin[("finE", k)] - LAT - _FMARGIN, rel), ("f", k)))
    for k in range(len(B_SIZES)):
        ops.append((act_begin[("B", k)] - LAT, ("cB", k)))
    ops.sort(key=lambda x: x[0])
    return [op for _, op in ops]

POOL_STREAM = _build_pool_stream()

NCBUF = 4  # SP f32 ring (4-block tiles)
NBBUF = 3  # Pool->ACT fp8 ring
NEBUF = 4  # Pool->DVE bf16 ring (E)
NESQ = 4  # E squared-tile ring
NNBUF = 3  # Pool->DVE bf16 ring (N)
NNSQ = 2  # N squared-tile ring

CB_BLKS = 4  # C tiles are at most 4 blocks


TREE_LEVELS_FULL = [480, 240, 120, 60, 30, 15]
TREE_LEVELS_HALF = [240, 120, 60, 30, 15]


def _build_program() -> bass.Bass:
    nc = bass.Bass("TRN2", target_bir_lowering=False, debug=False)
    AF = mybir.ActivationFunctionType
    ALU = mybir.AluOpType
    AX = mybir.AxisListType

    xy = nc.dram_tensor("xy", [128, T], F32, kind="ExternalInput").ap()
    out = nc.dram_tensor("bs", [128, NBLK], F32, kind="ExternalOutput").ap()

    cbuf = [
        nc.alloc_sbuf_tensor(f"cb{i}", [128, CB_BLKS * BLK], F32).ap()
        for i in range(NCBUF)
    ]
    bbuf = [nc.alloc_sbuf_tensor(f"bb{i}", [128, USAMP], FP8).ap() for i in range(NBBUF)]
    ebuf = [nc.alloc_sbuf_tensor(f"eb{i}", [128, USAMP], BF16).ap() for i in range(NEBUF)]
    esq = [nc.alloc_sbuf_tensor(f"es{i}", [128, USAMP], BF16).ap() for i in range(NESQ)]
    nbuf = [nc.alloc_sbuf_tensor(f"nb{i}", [128, USAMP], BF16).ap() for i in range(NNBUF)]
    nsq = [nc.alloc_sbuf_tensor(f"ns{i}", [128, USAMP], BF16).ap() for i in range(NNSQ)]
    bs = nc.alloc_sbuf_tensor("bst", [128, NBLK], F32).ap()
    junk = nc.alloc_sbuf_tensor("junk", [128, BLK], BF16).ap()

    def blkview(ap, nb):
        return ap[:, 0 : nb * BLK].rearrange("p (n b) -> p n b", b=BLK)

    sp_spans = list(C_SPANS)

    with (
        nc.Block() as block,
        nc.semaphore("dmaC") as dmaC,
        nc.semaphore("dmaB") as dmaB,
        nc.semaphore("dmaE") as dmaE,
        nc.semaphore("dmaN") as dmaN,
        nc.semaphore("foldE") as foldE,
        nc.semaphore("spfree") as spfree,
        nc.semaphore("bfree") as bfree,
        nc.semaphore("esqs") as esqs,
        nc.semaphore("nsqs") as nsqs,
        nc.semaphore("actfin") as actfin,
        nc.semaphore("dvefin") as dvefin,
        nc.semaphore("outs") as outs,
    ):
        @block.sync
        def _(sync):
            for s, (blk0, nb) in enumerate(sp_spans):
                if s >= NCBUF:
                    sync.wait_ge(spfree, s - NCBUF + 1)
                off = blk0 * BLK
                sync.dma_start(
                    out=cbuf[s % NCBUF][:, 0 : nb * BLK], in_=xy[:, off : off + nb * BLK]
                ).then_inc(dmaC, 16)
            sync.wait_ge(actfin, 1)
            sync.wait_ge(dvefin, 1)
            sync.dma_start(out=out, in_=bs).then_inc(outs, 16)
            sync.wait_ge(outs, 16)

        @block.gpsimd
        def _(g):
            for kind, k in POOL_STREAM:
                if kind == "cB":
                    if k >= NBBUF:
                        g.wait_ge(bfree, k - NBBUF + 1)
                    blk0, nb = B_SPANS[k]
                    off = blk0 * BLK
                    g.dma_start(
                        out=bbuf[k % NBBUF][:, 0 : nb * BLK],
                        in_=xy[:, off : off + nb * BLK],
                    ).then_inc(dmaB, 16)
                elif kind == "cE":
                    if k >= NEBUF:
                        g.wait_ge(esqs, k - NEBUF + 1)
                    blk0, nb = E_SPANS[k]
                    off = blk0 * BLK
                    g.dma_start(
                        out=ebuf[k % NEBUF][:, 0 : nb * BLK],
                        in_=xy[:, off : off + nb * BLK],
                    ).then_inc(dmaE, 16)
                elif kind == "cN":
                    if k >= NNBUF:
                        g.wait_ge(nsqs, k - NNBUF + 1)
                    blk0, nb = N_SPANS[k]
                    off = blk0 * BLK
                    g.dma_start(
                        out=nbuf[k % NNBUF][:, 0 : nb * BLK],
                        in_=xy[:, off : off + nb * BLK],
                    ).then_inc(dmaN, 16)
                else:  # fold: sq tile halves 960 -> 480, in place, accum add
                    g.wait_ge(esqs, k + 1)
                    v = blkview(esq[k % NESQ], E_SPANS[k][1])
                    g.dma_start(
                        out=v[:, :, 0:480], in_=v[:, :, 480:960], accum_op=ALU.add
                    ).then_inc(foldE, 16)

        @block.scalar
        def _(scalar):
            last = len(ACT_STREAM) - 1
            for pos, (kind, k) in enumerate(ACT_STREAM):
                if kind == "SP":
                    scalar.wait_ge(dmaC, 16 * (k + 1))
                    blk0, nb = sp_spans[k]
                    tile = cbuf[k % NCBUF]
                    for b in range(nb):
                        inst = scalar.activation(
                            junk,
                            tile[:, b * BLK : (b + 1) * BLK],
                            AF.Square,
                            accum_out=bs[:, blk0 + b : blk0 + b + 1],
                        )
                    inst.then_inc(spfree, 1)
                else:  # B unit
                    scalar.wait_ge(dmaB, 16 * (k + 1))
                    blk0, nb = B_SPANS[k]
                    tile = bbuf[k % NBBUF]
                    for b in range(nb):
                        inst = scalar.activation(
                            junk,
                            tile[:, b * BLK : (b + 1) * BLK],
                            AF.Square,
                            accum_out=bs[:, blk0 + b : blk0 + b + 1],
                        )
                    inst.then_inc(bfree, 1)
                if pos == last:
                    scalar.drain().then_inc(actfin, 1)

        @block.vector
        def _(vector):
            lp = nc.allow_low_precision

            def tree(v, levels, cols, nb):
                # Chained in-place tree levels are unsafe on the device
                # (same-engine RAW through the DVE write pipe); a single
                # sequential reduce never outruns the producer, so reduce.
                return vector.reduce_sum(cols, v[:, :, 0 : 2 * levels[0]], axis=AX.X)

            last = len(DVE_STREAM) - 1
            for pos, (kind, k) in enumerate(DVE_STREAM):
                if kind == "sqE":
                    vector.wait_ge(dmaE, 16 * (k + 1))
                    nb = E_SPANS[k][1]
                    with lp("bf16 squares"):
                        inst = vector.tensor_tensor(
                            esq[k % NESQ][:, 0 : nb * BLK],
                            ebuf[k % NEBUF][:, 0 : nb * BLK],
                            ebuf[k % NEBUF][:, 0 : nb * BLK],
                            op=ALU.mult,
                        )
                    inst.then_inc(esqs, 1)
                elif kind == "finE":
                    vector.wait_ge(foldE, 16 * (k + 1))
                    blk0, nb = E_SPANS[k]
                    inst = tree(
                        blkview(esq[k % NESQ], nb),
                        TREE_LEVELS_HALF,
                        bs[:, blk0 : blk0 + nb],
                        nb,
                    )
                elif kind == "N":
                    vector.wait_ge(dmaN, 16 * (k + 1))
                    blk0, nb = N_SPANS[k]
                    with lp("bf16 squares"):
                        vector.tensor_tensor(
                            nsq[k % NNSQ][:, 0 : nb * BLK],
                            nbuf[k % NNBUF][:, 0 : nb * BLK],
                            nbuf[k % NNBUF][:, 0 : nb * BLK],
                            op=ALU.mult,
                        ).then_inc(nsqs, 1)
                    inst = tree(
                        blkview(nsq[k % NNSQ], nb),
                        TREE_LEVELS_FULL,
                        bs[:, blk0 : blk0 + nb],
                        nb,
                    )
                if pos == last:
                    inst.then_inc(dvefin, 1)

    return nc


def make_in_maps(x_env: np.ndarray, y_env: np.ndarray) -> list[dict[str, np.ndarray]]:
    x = np.asarray(x_env, dtype=np.float32).reshape(ROWS, T)
    y = np.asarray(y_env, dtype=np.float32).reshape(ROWS, T)
    in_maps = []
    for i in range(N_CORES):
        shard = np.concatenate(
            [x[i * RPC : (i + 1) * RPC], y[i * RPC : (i + 1) * RPC]], axis=0
        )
        in_maps.append({"xy": np.ascontiguousarray(shard)})
    return in_maps


def lufs_from_bs(bs: np.ndarray) -> np.ndarray:
    """Per-row gated lufs from [N, 200] block energy sums (float64 host math)."""
    bs = np.asarray(bs, dtype=np.float64)
    n = bs.shape[0]
    # frame f = blocks 3f..3f+9; cumulative sum gives all frame windows
    cs = np.concatenate([np.zeros((n, 1)), np.cumsum(bs, axis=1)], axis=1)
    starts = 3 * np.arange(NFRM)
    z = (cs[:, starts + 10] - cs[:, starts]) / FRAME  # [N, 64]
    el = -0.691 + 10.0 * np.log10(z + EPS)
    idx_a = (el > GAMMA_A).astype(np.float64)
    z_ave_a = (z * idx_a).sum(1, keepdims=True) / (idx_a.sum(1, keepdims=True) + EPS)
    gamma_r = -0.691 + 10.0 * np.log10(z_ave_a + EPS) - 10.0
    idx_ar = idx_a * (el > gamma_r)
    z_ave_ar = (z * idx_ar).sum(1, keepdims=True) / (idx_ar.sum(1, keepdims=True) + EPS)
    return (-0.691 + 10.0 * np.log10(z_ave_ar + EPS)).reshape(n)


def finish(per_core_bs: list[np.ndarray]) -> np.ndarray:
    total = 0.0
    for bsc in per_core_bs:
        lufs = lufs_from_bs(np.asarray(bsc).reshape(128, NBLK))
        total += np.maximum(lufs[RPC:] - lufs[:RPC], 0.0).sum()
    return np.array(ALPHA * total, dtype=np.float32)


def kernel(x_env: np.ndarray, y_env: np.ndarray) -> np.ndarray:
    nc = _build_program()
    in_maps = make_in_maps(x_env, y_env)
    res = run_bass_kernel_spmd(nc, in_maps, core_ids=list(range(N_CORES)))
    return finish([res.results[i]["bs"] for i in range(N_CORES)])
